# revision 1
# baseline (speedup 1.0000x reference)
"""Causal self-attention on 8 trn2 NeuronCores.

Sharding: core = 2*b + g  (b in 0..3 batches, g in 0..1 head-groups of 8
heads). Each core computes, for its batch b and its 8 heads:
  qkv^T = (x_b @ Wqkv_slice)^T   (feature-major, pair-interleaved cols)
  per-head causal softmax attention (scores^T layout, ones-augmented V
  accumulates the softmax denominator in the same matmul)
  partial out^T = y^T-scaled @ Wp_slice  -> [1024, 2048]
Host gathers: out[b] = (partial[2b] + partial[2b+1]).T + b_proj.

All big matmuls run in fp32r (hw-measured rel err ~1.6e-4).
"""

import numpy as np
import ml_dtypes

B, T, E, H = 4, 2048, 1024, 16
HD = E // H  # 64

_CACHE = {}


def _build():
    from contextlib import ExitStack

    import concourse.bass as bass
    import concourse.mybir as mybir
    import concourse.tile as tile
    from concourse import bacc
    from concourse.masks import make_identity

    F32 = mybir.dt.float32
    F32R = mybir.dt.float32r
    BF16 = mybir.dt.bfloat16
    AF = mybir.ActivationFunctionType
    MUL = mybir.AluOpType.mult

    nc = bacc.Bacc("TRN2", target_bir_lowering=False)
    xin = nc.dram_tensor("xin", [T, E], BF16, kind="ExternalInput")
    wqkv = nc.dram_tensor("wqkv", [128, 8, 1536], BF16, kind="ExternalInput")
    bqkv = nc.dram_tensor("bqkv", [128, 12], F32, kind="ExternalInput")
    wp = nc.dram_tensor("wp", [128, 4, 1024], BF16, kind="ExternalInput")
    outT = nc.dram_tensor("outT", [E, T], F32, kind="ExternalOutput")

    with tile.TileContext(nc) as tc, ExitStack() as ctx:
        const = ctx.enter_context(tc.tile_pool(name="const", bufs=1))
        ident32 = const.tile([128, 128], F32, tag="ident32")
        make_identity(nc, ident32[:])
        identr = const.tile([128, 128], BF16, tag="identr")
        nc.vector.tensor_copy(identr[:], ident32[:])
        # stacked 64x64 identities at partition 0 and 64 (for v-transpose,
        # whose lhsT sits at partition base 0 or 64)
        id2f = const.tile([128, 64], F32, tag="id2f")
        nc.gpsimd.memset(id2f[:], 0.0)
        for off in (0, 64):
            nc.gpsimd.affine_select(
                out=id2f[:],
                in_=id2f[:],
                compare_op=mybir.AluOpType.not_equal,
                fill=1.0,
                base=-off,
                pattern=[[-1, 64]],
                channel_multiplier=1,
            )
        id2 = const.tile([128, 64], BF16, tag="id2")
        nc.vector.tensor_copy(id2[:], id2f[:])
        masks = []
        for j in range(4):
            mjf = const.tile([128, 512], F32, tag=f"maskf{j}", name=f"maskf{j}")
            nc.gpsimd.memset(mjf[:], 1.0)
            nc.gpsimd.affine_select(
                out=mjf[:],
                in_=mjf[:],
                compare_op=mybir.AluOpType.is_ge,
                fill=0.0,
                base=-128 * j,
                pattern=[[1, 512]],
                channel_multiplier=-1,
            )
            mj = const.tile([128, 512], BF16, tag=f"mask{j}", name=f"mask{j}")
            nc.vector.tensor_copy(mj[:], mjf[:])
            masks.append(mj)
        biasT = const.tile([128, 12], F32, tag="biasT")
        nc.sync.dma_start(biasT[:], bqkv[:])

        qkvT_pool = ctx.enter_context(tc.tile_pool(name="qkvT", bufs=1))
        qkvT = qkvT_pool.tile([128, 12, T], BF16, tag="qkvT")

        # ---------------- Phase A: x^T + QKV projection ----------------
        with (
            tc.tile_pool(name="xstage", bufs=3) as xstage_pool,
            tc.tile_pool(name="xT", bufs=1) as xT_pool,
            tc.tile_pool(name="wq", bufs=3) as wq_pool,
            tc.tile_pool(name="psAt", bufs=2, space="PSUM") as psAt,
            tc.tile_pool(name="psAm", bufs=2, space="PSUM") as psAm,
        ):
            for half in range(2):
                t0 = half * 1024
                xTh = xT_pool.tile([128, 8, 1024], BF16, tag="xTh")
                for tt in range(8):
                    xs = xstage_pool.tile([128, E], BF16, tag="xs")
                    nc.sync.dma_start(xs[:], xin[t0 + tt * 128 : t0 + (tt + 1) * 128, :])
                    for k in range(8):
                        pt = psAt.tile([128, 128], F32, tag="ptr")
                        nc.tensor.matmul(
                            pt[:],
                            xs[:, k * 128 : (k + 1) * 128],
                            identr[:],
                            start=True,
                            stop=True,
                        )
                        nc.vector.tensor_copy(
                            xTh[:, k, tt * 128 : (tt + 1) * 128], pt[:]
                        )
                for m in range(12):
                    wqm = wq_pool.tile([128, 8, 128], BF16, tag="wqm")
                    nc.sync.dma_start(wqm[:], wqkv[:, :, m * 128 : (m + 1) * 128])
                    pq = psAm.tile([128, 1024], F32, tag="pq")
                    for k in range(8):
                        for j in range(2):
                            nc.tensor.matmul(
                                pq[:, j * 512 : (j + 1) * 512],
                                wqm[:, k, :],
                                xTh[:, k, j * 512 : (j + 1) * 512],
                                start=(k == 0),
                                stop=(k == 7),
                            )
                    nc.vector.tensor_scalar_add(
                        qkvT[:, m, t0 : t0 + 1024], pq[:], biasT[:, m : m + 1]
                    )

        # ---------------- Phase B: attention ----------------
        yT_pool = ctx.enter_context(tc.tile_pool(name="yT", bufs=1))
        yT = yT_pool.tile([128, 4, T], BF16, tag="yT")

        with (
            tc.tile_pool(name="vaug", bufs=2) as vaug_pool,
            tc.tile_pool(name="Pp", bufs=4) as P_pool,
            tc.tile_pool(name="smallB", bufs=3) as smallB,
            tc.tile_pool(name="psBs", bufs=3, space="PSUM") as psBs,
            tc.tile_pool(name="psBy", bufs=1, space="PSUM") as psBy,
        ):
            for p in range(4):
                vaug = vaug_pool.tile([128, 2, 16 * 65], BF16, tag="vaug")
                nc.gpsimd.memset(vaug[:], 1.0)
                for s in range(2):
                    for kb in range(16):
                        pv = psBs.tile([128, 1024], F32, tag="s")
                        nc.tensor.matmul(
                            pv[:, 0:64],
                            qkvT[64 * s : 64 * s + 64, 3 * p + 2, kb * 128 : (kb + 1) * 128],
                            id2[64 * s : 64 * s + 64, :],
                            start=True,
                            stop=True,
                            tile_position=(64 * s, 0),
                        )
                        nc.vector.tensor_copy(
                            vaug[:, s, kb * 65 : kb * 65 + 64], pv[:, 0:64]
                        )
                for s in range(2):
                    qT = qkvT[64 * s : 64 * s + 64, 3 * p, :]
                    kT = qkvT[64 * s : 64 * s + 64, 3 * p + 1, :]
                    for qc in range(2):
                        ymm = psBy.tile([128, 1024], F32, tag="y")
                        kmax = (qc + 1) * 8
                        # last-writer kb per 512-chunk of this q block
                        klast = [
                            min(kmax - 1, (qc * 2 + ci + 1) * 4 - 1) for ci in range(2)
                        ]
                        for kb in range(kmax):
                            diag = kb >= qc * 8
                            q_lo = (
                                qc * 1024 if not diag else (kb * 128 // 512) * 512
                            )
                            w = (qc + 1) * 1024 - q_lo
                            sp = psBs.tile([128, 1024], F32, tag="s")
                            for j in range(w // 512):
                                nc.tensor.matmul(
                                    sp[:, j * 512 : (j + 1) * 512],
                                    kT[:, kb * 128 : (kb + 1) * 128],
                                    qT[:, q_lo + j * 512 : q_lo + (j + 1) * 512],
                                    start=True,
                                    stop=True,
                                )
                            Pt = P_pool.tile([128, 1024], BF16, tag="P")
                            nc.scalar.activation(
                                Pt[:, :w], sp[:, :w], AF.Exp, scale=0.125
                            )
                            if diag:
                                nc.gpsimd.tensor_mul(
                                    Pt[:, 0:512], Pt[:, 0:512], masks[kb % 4][:]
                                )
                            for j in range(w // 512):
                                col = q_lo - qc * 1024 + j * 512
                                ci = col // 512
                                nc.tensor.matmul(
                                    ymm[0:65, col : col + 512],
                                    vaug[:, s, kb * 65 : kb * 65 + 65],
                                    Pt[:, j * 512 : (j + 1) * 512],
                                    start=(kb == 0),
                                    stop=(kb == klast[ci]),
                                )
                        # normalize: y = ymm[0:64] * (1/denom) broadcast
                        rec = smallB.tile([1, 1024], F32, tag="rec")
                        nc.vector.reciprocal(rec[0:1, :], ymm[64:65, :])
                        bcs = smallB.tile([64, 1024], F32, tag="bcs")
                        nc.gpsimd.partition_broadcast(bcs[:], rec[0:1, :])
                        nc.vector.tensor_tensor(
                            out=yT[64 * s : 64 * s + 64, p, qc * 1024 : (qc + 1) * 1024],
                            in0=ymm[0:64, :],
                            in1=bcs[:],
                            op=MUL,
                        )

        # ---------------- Phase C: output projection ----------------
        with (
            tc.tile_pool(name="wpp", bufs=1) as wp_pool,
            tc.tile_pool(name="ob", bufs=2) as ob_pool,
            tc.tile_pool(name="psC", bufs=8, space="PSUM") as psC,
        ):
            wps = wp_pool.tile([128, 4, 1024], BF16, tag="wps")
            nc.sync.dma_start(wps[:], wp[:])
            for m in range(8):
                pn = [
                    psC.tile([128, 512], F32, tag="pc", name=f"pc{m}_{n}")
                    for n in range(4)
                ]
                for k in range(4):
                    for n in range(4):
                        nc.tensor.matmul(
                            pn[n][:],
                            wps[:, k, m * 128 : (m + 1) * 128],
                            yT[:, k, n * 512 : (n + 1) * 512],
                            start=(k == 0),
                            stop=(k == 3),
                        )
                ob = ob_pool.tile([128, T], F32, tag="ob")
                for n in range(4):
                    nc.scalar.copy(ob[:, n * 512 : (n + 1) * 512], pn[n][:])
                nc.sync.dma_start(outT[m * 128 : (m + 1) * 128, :], ob[:])

    nc.compile()
    return nc


def _get_nc():
    if "nc" not in _CACHE:
        _CACHE["nc"] = _build()
    return _CACHE["nc"]


def _prep_core_inputs(x, w_attn, b_attn, w_proj, b, g):
    cols = []
    for p in range(4):
        off = 512 * g + 128 * p
        cols += [
            w_attn[:, off : off + 128],
            w_attn[:, E + off : E + off + 128],
            w_attn[:, 2 * E + off : 2 * E + off + 128],
        ]
    wq = np.concatenate(cols, axis=1)  # [1024, 1536]
    wq = np.ascontiguousarray(
        wq.reshape(8, 128, 1536).transpose(1, 0, 2), dtype=np.float32
    )
    bcols = []
    for p in range(4):
        off = 512 * g + 128 * p
        bcols += [
            b_attn[off : off + 128],
            b_attn[E + off : E + off + 128],
            b_attn[2 * E + off : 2 * E + off + 128],
        ]
    bq = np.stack(bcols, axis=1).astype(np.float32)  # [128, 12]
    wpr = np.concatenate(
        [w_proj[512 * g + 128 * p : 512 * g + 128 * p + 128, :] for p in range(4)],
        axis=0,
    )  # [512, 1024]
    wpr = np.ascontiguousarray(
        wpr.reshape(4, 128, 1024).transpose(1, 0, 2), dtype=np.float32
    )
    return {
        "xin": np.ascontiguousarray(x[b]).astype(ml_dtypes.bfloat16),
        "wqkv": wq.astype(ml_dtypes.bfloat16),
        "bqkv": np.ascontiguousarray(bq),
        "wp": wpr.astype(ml_dtypes.bfloat16),
    }


def kernel(x, w_attn, b_attn, w_proj, b_proj, _trace=False):
    from concourse.bass_utils import run_bass_kernel_spmd

    x = np.asarray(x, dtype=np.float32)
    w_attn = np.asarray(w_attn, dtype=np.float32)
    b_attn = np.asarray(b_attn, dtype=np.float32)
    w_proj = np.asarray(w_proj, dtype=np.float32)
    b_proj = np.asarray(b_proj, dtype=np.float32)

    nc = _get_nc()
    in_maps = [
        _prep_core_inputs(x, w_attn, b_attn, w_proj, core // 2, core % 2)
        for core in range(8)
    ]
    res = run_bass_kernel_spmd(
        nc, in_maps, core_ids=list(range(8)), trace=_trace
    )
    _CACHE["last_results"] = res
    out = np.empty((B, T, E), dtype=np.float32)
    for b in range(B):
        acc = res.results[2 * b]["outT"] + res.results[2 * b + 1]["outT"]
        out[b] = acc.T + b_proj[None, :]
    return out



# revision 10
# speedup vs baseline: 1.5465x; 1.5465x over previous
"""Causal self-attention on 8 trn2 NeuronCores.

Sharding: core = 2*b + g  (b in 0..3 batches, g in 0..1 head-groups of 8
heads). Each core computes, for its batch b and its 8 heads:
  qkv^T = Wqkv_slice^T @ x^T   (x^T pre-transposed on host)
  per-head causal softmax attention (scores^T layout; ones-augmented V
  accumulates the softmax denominator in the same matmul; causal mask
  applied by accumulating a -30000 triangle into the score PSUM via PE)
  partial out^T = y^T-scaled @ Wp_slice  -> [1024, 2048] bf16
Host gathers: out[b] = (partial[2b] + partial[2b+1]).T + b_proj.

QKV projection for head-pair p+1 is interleaved into the attention
pair-iterations of head-pair p so the PE never idles during softmax.
"""

import numpy as np
import ml_dtypes

B, T, E, H = 4, 2048, 1024, 16
HD = E // H  # 64
NEG = -30000.0

_CACHE = {}


def _build():
    from contextlib import ExitStack

    import concourse.bass as bass
    import concourse.mybir as mybir
    import concourse.tile as tile
    from concourse import bacc
    from concourse.masks import make_identity

    F32 = mybir.dt.float32
    BF16 = mybir.dt.bfloat16
    AF = mybir.ActivationFunctionType
    MUL = mybir.AluOpType.mult

    nc = bacc.Bacc("TRN2", target_bir_lowering=False)
    xT = nc.dram_tensor("xT", [128, 8, T], BF16, kind="ExternalInput")
    wqkv = nc.dram_tensor("wqkv", [128, 8, 1536], BF16, kind="ExternalInput")
    bqkv = nc.dram_tensor("bqkv", [128, 12], F32, kind="ExternalInput")
    wp = nc.dram_tensor("wp", [128, 4, 1024], BF16, kind="ExternalInput")
    outT = nc.dram_tensor("outT", [E, T], BF16, kind="ExternalOutput")

    with tile.TileContext(nc) as tc, ExitStack() as ctx:
        const = ctx.enter_context(tc.tile_pool(name="const", bufs=1))
        ident32 = const.tile([128, 128], F32, tag="ident32")
        make_identity(nc, ident32[:])
        identr = const.tile([128, 128], BF16, tag="identr")
        nc.vector.tensor_copy(identr[:], ident32[:])
        # stacked 64x64 identities at partition 0 and 64 (for v-transpose,
        # whose lhsT sits at partition base 0 or 64)
        id2f = const.tile([128, 64], F32, tag="id2f")
        nc.gpsimd.memset(id2f[:], 0.0)
        for off in (0, 64):
            nc.gpsimd.affine_select(
                out=id2f[:],
                in_=id2f[:],
                compare_op=mybir.AluOpType.not_equal,
                fill=1.0,
                base=-off,
                pattern=[[-1, 64]],
                channel_multiplier=1,
            )
        id2 = const.tile([128, 64], BF16, tag="id2")
        nc.vector.tensor_copy(id2[:], id2f[:])
        # additive causal masks: Mw[d] is [128, 128*(d+1)];
        # Mw[d][ch, c] = 0 where c >= ch + 128*d else NEG.
        # For a diagonal key-block kb (d = kb - 4*qc), accumulating Mw[d]
        # into score cols [0, 128*(d+1)) makes exp() zero the masked region.
        masks = []
        for d in range(4):
            w = 128 * (d + 1)
            mjf = const.tile([128, 512], F32, tag=f"maskf{d}", name=f"maskf{d}")
            nc.gpsimd.memset(mjf[:], 0.0)
            nc.gpsimd.affine_select(
                out=mjf[:, :w],
                in_=mjf[:, :w],
                compare_op=mybir.AluOpType.is_ge,
                fill=NEG,
                base=-128 * d,
                pattern=[[1, w]],
                channel_multiplier=-1,
            )
            mj = const.tile([128, 512], BF16, tag=f"mask{d}", name=f"mask{d}")
            nc.vector.tensor_copy(mj[:], mjf[:])
            masks.append(mj)
        biasT = const.tile([128, 12], F32, tag="biasT")
        nc.sync.dma_start(biasT[:], bqkv[:])

        big = ctx.enter_context(tc.tile_pool(name="big", bufs=1))
        xTs = big.tile([128, 8, T], BF16, tag="xTs")
        qkvT = big.tile([128, 12, T], BF16, tag="qkvT")
        yT = big.tile([128, 4, T], BF16, tag="yT")
        for k in range(8):
            nc.sync.dma_start(xTs[:, k, :], xT[:, k, :])

        ps = ctx.enter_context(tc.tile_pool(name="ps", bufs=1, space="PSUM"))
        wq_pool = ctx.enter_context(tc.tile_pool(name="wqp", bufs=3))
        vaug_pool = ctx.enter_context(tc.tile_pool(name="vaugp", bufs=2))
        pt_pool = ctx.enter_context(tc.tile_pool(name="ptp", bufs=2))
        sm_pool = ctx.enter_context(tc.tile_pool(name="smp", bufs=3))
        ob_pool = ctx.enter_context(tc.tile_pool(name="obp", bufs=2))

        state = {"wqm": {}, "vaug": {}}

        def emit_dma(m):
            wqm = wq_pool.tile([128, 8, 128], BF16, tag="wqm", name=f"wqm{m}")
            nc.sync.dma_start(wqm[:], wqkv[:, :, m * 128 : (m + 1) * 128])
            state["wqm"][m] = wqm

        def emit_mm(m, j):
            wqm = state["wqm"][m]
            pq = ps.tile([128, 512], F32, tag="pq", bufs=2, name=f"pq{m}_{j}")
            for k in range(8):
                nc.tensor.matmul(
                    pq[:],
                    wqm[:, k, :],
                    xTs[:, k, j * 512 : (j + 1) * 512],
                    start=(k == 0),
                    stop=(k == 7),
                )
            nc.vector.tensor_scalar_add(
                qkvT[:, m, j * 512 : (j + 1) * 512], pq[:], biasT[:, m : m + 1]
            )

        def emit_vtrans(p, s, half):
            # transpose v for 8 key blocks into vaug (key-major, 128-stride;
            # cols 64-127 stay 1.0 so the PV matmul replicates the softmax
            # denominator across psum partitions 64-127)
            vaug = state["vaug"][p]
            pv = ps.tile([128, 512], F32, tag="pq", bufs=2, name=f"pv{p}_{s}_{half}")
            for i in range(8):
                kb = half * 8 + i
                nc.tensor.matmul(
                    pv[:, i * 64 : (i + 1) * 64],
                    qkvT[64 * s : 64 * s + 64, 3 * p + 2, kb * 128 : (kb + 1) * 128],
                    id2[64 * s : 64 * s + 64, :],
                    start=True,
                    stop=True,
                    tile_position=(64 * s, 0),
                )
            nc.vector.tensor_copy(
                vaug[:, s, half * 8 : half * 8 + 8, 0:64],
                pv[:].rearrange("p (i c) -> p i c", i=8),
            )

        def emit_vaug_alloc(p):
            vaug = vaug_pool.tile([128, 2, 16, 72], BF16, tag="vaug", name=f"vaug{p}")
            nc.gpsimd.memset(vaug[:], 1.0)
            state["vaug"][p] = vaug

        def qkv_quanta(p):
            for ml in range(3):
                yield ("dma", 3 * p + ml)
                for j in range(4):
                    yield ("mm", 3 * p + ml, j)
            yield ("vaug", p)
            for s in range(2):
                for half in range(2):
                    yield ("vtrans", p, s, half)

        def run_quantum(q):
            if q[0] == "dma":
                emit_dma(q[1])
            elif q[0] == "mm":
                emit_mm(q[1], q[2])
            elif q[0] == "vaug":
                emit_vaug_alloc(q[1])
            else:
                emit_vtrans(q[1], q[2], q[3])

        def attention(p, filler):
            vaug = state["vaug"][p]
            for qc in range(4):
                kmax = 4 * qc + 4
                ym = {}
                for s in range(2):
                    ym[s] = ps.tile(
                        [128, 512], F32, tag=f"ym{s}", bufs=1, name=f"ym{p}_{qc}_{s}"
                    )
                for t in range(kmax // 2):
                    sc = {}
                    pt = {}
                    for s in range(2):
                        sc[s] = ps.tile(
                            [128, 1024],
                            F32,
                            tag=f"sc{s}",
                            bufs=1,
                            name=f"sc{p}_{qc}_{t}_{s}",
                        )
                        qT = qkvT[64 * s : 64 * s + 64, 3 * p, qc * 512 : qc * 512 + 512]
                        kT = qkvT[64 * s : 64 * s + 64, 3 * p + 1, :]
                        for i in range(2):
                            kb = 2 * t + i
                            d = kb - 4 * qc
                            nc.tensor.matmul(
                                sc[s][:, i * 512 : (i + 1) * 512],
                                kT[:, kb * 128 : (kb + 1) * 128],
                                qT,
                                start=True,
                                stop=(d < 0),
                            )
                            if d >= 0:
                                w = 128 * (d + 1)
                                nc.tensor.matmul(
                                    sc[s][:, i * 512 : i * 512 + w],
                                    identr[:],
                                    masks[d][:, :w],
                                    start=False,
                                    stop=True,
                                )
                    # PE filler while the scalar engine runs exp
                    if filler:
                        run_quantum(filler.pop(0))
                    for s in range(2):
                        pt[s] = pt_pool.tile(
                            [128, 1024],
                            BF16,
                            tag=f"pt{s}",
                            name=f"pt{p}_{qc}_{t}_{s}",
                        )
                        nc.scalar.activation(pt[s][:], sc[s][:], AF.Exp, scale=0.125)
                    for s in range(2):
                        for i in range(2):
                            kb = 2 * t + i
                            c0 = max(0, 128 * (kb - 4 * qc))
                            nc.tensor.matmul(
                                ym[s][:, c0:512],
                                vaug[:, s, kb, :],
                                pt[s][:, i * 512 + c0 : (i + 1) * 512],
                                start=(kb == 0),
                                stop=(kb == kmax - 1),
                            )
                # normalize: yT = ym[0:64] * 1/ym[64:128] (denominator was
                # replicated across partitions 64-127 by the ones columns)
                for s in range(2):
                    rec = sm_pool.tile([64, 512], F32, tag="rec", name=f"rec{p}{qc}{s}")
                    nc.vector.reciprocal(rec[:], ym[s][64:128, :])
                    nc.vector.tensor_tensor(
                        out=yT[64 * s : 64 * s + 64, p, qc * 512 : qc * 512 + 512],
                        in0=ym[s][0:64, :],
                        in1=rec[:],
                        op=MUL,
                    )

        # ---------------- main schedule ----------------
        for q in qkv_quanta(0):
            run_quantum(q)
        for p in range(4):
            filler = list(qkv_quanta(p + 1)) if p < 3 else []
            attention(p, filler)

        # ---------------- output projection ----------------
        wps = big.tile([128, 4, 1024], BF16, tag="wps")
        nc.sync.dma_start(wps[:], wp[:])
        for m in range(8):
            ob = ob_pool.tile([128, T], BF16, tag="ob", name=f"ob{m}")
            for n in range(4):
                pn = ps.tile([128, 512], F32, tag="pq", bufs=2, name=f"pc{m}_{n}")
                for k in range(4):
                    nc.tensor.matmul(
                        pn[:],
                        wps[:, k, m * 128 : (m + 1) * 128],
                        yT[:, k, n * 512 : (n + 1) * 512],
                        start=(k == 0),
                        stop=(k == 3),
                    )
                nc.vector.tensor_copy(ob[:, n * 512 : (n + 1) * 512], pn[:])
            nc.sync.dma_start(outT[m * 128 : (m + 1) * 128, :], ob[:])

    nc.compile()
    return nc


def _get_nc():
    if "nc" not in _CACHE:
        _CACHE["nc"] = _build()
    return _CACHE["nc"]


def _prep_core_inputs(xTb, w_attn, b_attn, w_proj, g):
    cols = []
    for p in range(4):
        off = 512 * g + 128 * p
        cols += [
            w_attn[:, off : off + 128],
            w_attn[:, E + off : E + off + 128],
            w_attn[:, 2 * E + off : 2 * E + off + 128],
        ]
    wq = np.concatenate(cols, axis=1)  # [1024, 1536]
    wq = np.ascontiguousarray(
        wq.reshape(8, 128, 1536).transpose(1, 0, 2), dtype=np.float32
    )
    bcols = []
    for p in range(4):
        off = 512 * g + 128 * p
        bcols += [
            b_attn[off : off + 128],
            b_attn[E + off : E + off + 128],
            b_attn[2 * E + off : 2 * E + off + 128],
        ]
    bq = np.stack(bcols, axis=1).astype(np.float32)  # [128, 12]
    wpr = np.concatenate(
        [w_proj[512 * g + 128 * p : 512 * g + 128 * p + 128, :] for p in range(4)],
        axis=0,
    )  # [512, 1024]
    wpr = np.ascontiguousarray(
        wpr.reshape(4, 128, 1024).transpose(1, 0, 2), dtype=np.float32
    )
    return {
        "xT": xTb,
        "wqkv": wq.astype(ml_dtypes.bfloat16),
        "bqkv": np.ascontiguousarray(bq),
        "wp": wpr.astype(ml_dtypes.bfloat16),
    }


def kernel(x, w_attn, b_attn, w_proj, b_proj, _trace=False):
    from concourse.bass_utils import run_bass_kernel_spmd

    x = np.asarray(x, dtype=np.float32)
    w_attn = np.asarray(w_attn, dtype=np.float32)
    b_attn = np.asarray(b_attn, dtype=np.float32)
    w_proj = np.asarray(w_proj, dtype=np.float32)
    b_proj = np.asarray(b_proj, dtype=np.float32)

    nc = _get_nc()
    xTs = []
    for b in range(B):
        xTb = np.ascontiguousarray(x[b].T).astype(ml_dtypes.bfloat16)
        xTs.append(
            np.ascontiguousarray(xTb.reshape(8, 128, T).transpose(1, 0, 2))
        )
    in_maps = [
        _prep_core_inputs(xTs[core // 2], w_attn, b_attn, w_proj, core % 2)
        for core in range(8)
    ]
    res = run_bass_kernel_spmd(nc, in_maps, core_ids=list(range(8)), trace=_trace)
    _CACHE["last_results"] = res
    out = np.empty((B, T, E), dtype=np.float32)
    for b in range(B):
        acc = res.results[2 * b]["outT"].astype(np.float32) + res.results[
            2 * b + 1
        ]["outT"].astype(np.float32)
        out[b] = acc.T + b_proj[None, :]
    return out
